# revision 1
# baseline (speedup 1.0000x reference)
# kernel.py — nn_CustomLinearEval: group-dequantized linear layer on 8 trn2 cores.
#
# out[b,s,n] = sum_k x[b,s,k] * w_dq[k,n] + bias[n]
#   w_dq = round(weight.T / s) * s,  s = step_scales[g,n] + 1e-8, g = k // 128
#
# Sharding: data-parallel over M = B*S (8 x 1024 rows). Each core:
#   - transposes its x shard on the PE (fp32, 128x128 tiles) into SBUF-resident x^T
#   - streams the full weight in natural [n,k] layout, dequantizes on DVE
#     (round-half-even via the +/-1.5*2^23 magic-number trick, matching jnp.round),
#     transposes each [n,k] tile to [k,n] on the PE
#   - accumulates out^T[n_tile=128, 1024] in PSUM over 32 k-tiles with
#     float32r matmuls (free dim 512)
#   - fuses bias-add into the PSUM->SBUF copy on the scalar engine
# Host gathers the 8 out^T shards and transposes once in numpy.

import numpy as np

GS = 128
EPS = 1e-8
B, S, K, N = 4, 2048, 4096, 4096
M = B * S
NCORES = 8
ML = M // NCORES          # 1024 rows of x per core
G = K // GS               # 32 quant groups
NT = N // 128             # 32 n tiles
KT = K // 128             # 32 k tiles
MT = ML // 128            # 8 m tiles per core
MAGIC = float(np.float32(12582912.0))  # 1.5 * 2**23: fp32 round-to-nearest-even trick

_NC_CACHE = {}


def _build_nc():
    import concourse.bass as bass
    import concourse.mybir as mybir
    import concourse.tile as tile

    f32 = mybir.dt.float32
    f32r = mybir.dt.float32r
    AF = mybir.ActivationFunctionType
    OP = mybir.AluOpType

    nc = bass.Bass()
    # x_t: host-pre-transposed x shard, [K, ML] (pure layout transform on host)
    x_t = nc.dram_tensor("x_t", [K, ML], f32r, kind="ExternalInput")
    w = nc.dram_tensor("w", [N, K], f32, kind="ExternalInput")
    srep = nc.dram_tensor("srep", [128, NT * G], f32, kind="ExternalInput")
    rrep = nc.dram_tensor("rrep", [128, NT * G], f32, kind="ExternalInput")
    brep = nc.dram_tensor("brep", [128, NT], f32, kind="ExternalInput")
    ident = nc.dram_tensor("ident", [128, 128], f32r, kind="ExternalInput")
    out_t = nc.dram_tensor("out_t", [N, ML], f32, kind="ExternalOutput")

    KH = K // 2  # stage x and w row-blocks in two 1 MiB halves

    with tile.TileContext(nc) as tc:
        with (
            tc.tile_pool(name="const", bufs=1) as constp,
            tc.tile_pool(name="xT", bufs=1) as xTp,
            tc.tile_pool(name="wnat", bufs=2) as wnatp,
            tc.tile_pool(name="t1", bufs=6) as t1p,
            tc.tile_pool(name="wdq", bufs=6) as wdqp,
            tc.tile_pool(name="wdqT", bufs=10) as wdqTp,
            tc.tile_pool(name="outsb", bufs=2) as outp,
            tc.tile_pool(name="tp_ps", bufs=2, space="PSUM") as tpps,
            tc.tile_pool(name="acc_ps", bufs=3, space="PSUM") as accps,
        ):
            id_sb = constp.tile([128, 128], f32r)
            nc.sync.dma_start(id_sb[:], ident[:, :])
            s_sb = constp.tile([128, NT * G], f32)
            nc.sync.dma_start(s_sb[:], srep[:, :])
            r_sb = constp.tile([128, NT * G], f32)
            nc.sync.dma_start(r_sb[:], rrep[:, :])
            b_sb = constp.tile([128, NT], f32)
            nc.sync.dma_start(b_sb[:], brep[:, :])

            # First weight row-block ahead of the x^T bulk load so the dequant
            # pipeline starts immediately.
            wn_first = [
                wnatp.tile([128, KH], f32, tag="wnat", name=f"wn_first{i}")
                for i in range(2)
            ]
            nc.sync.dma_start(wn_first[0][:], w[0:128, 0:KH])

            # x^T resident: column block kt*ML holds x^T k-tile kt, DMA'd directly
            # from the host-pre-transposed shard.
            xT = xTp.tile([128, KT * ML], f32r)
            for kt in range(KT):
                nc.sync.dma_start(
                    xT[:, kt * ML : (kt + 1) * ML],
                    x_t[kt * 128 : (kt + 1) * 128, :],
                )
            nc.sync.dma_start(wn_first[1][:], w[0:128, KH:K])

            # ---- main: per n-tile dequant + transpose + matmul ----
            xT_r = xT[:]
            for nt in range(NT):
                acc = accps.tile([128, ML], f32)
                for h in range(2):
                    if nt == 0:
                        wn = wn_first[h]
                    else:
                        wn = wnatp.tile([128, KH], f32, tag="wnat")
                        nc.sync.dma_start(
                            wn[:], w[nt * 128 : (nt + 1) * 128, h * KH : (h + 1) * KH]
                        )
                    for j in range(4):  # 4 batches of 4 k-tiles
                        ps = tpps.tile([128, 512], f32r)
                        wT = wdqTp.tile([128, 512], f32r)
                        for q in range(4):
                            kt = h * (KT // 2) + j * 4 + q
                            col = nt * G + kt
                            t1 = t1p.tile([128, 128], f32)
                            # t1 = (w * (1/s)) + MAGIC   (rounds half-even into integer bits)
                            nc.vector.tensor_scalar(
                                t1[:],
                                wn[:, (j * 4 + q) * 128 : (j * 4 + q + 1) * 128],
                                r_sb[:, col : col + 1],
                                MAGIC,
                                op0=OP.mult,
                                op1=OP.add,
                            )
                            # w_dq = (t1 - MAGIC) * s, rounded to fp32r on write
                            wdq = wdqp.tile([128, 128], f32r)
                            nc.vector.tensor_scalar(
                                wdq[:],
                                t1[:],
                                MAGIC,
                                s_sb[:, col : col + 1],
                                op0=OP.subtract,
                                op1=OP.mult,
                            )
                            nc.tensor.transpose(
                                ps[:, q * 128 : (q + 1) * 128], wdq[:], id_sb[:]
                            )
                        nc.scalar.copy(wT[:], ps[:])
                        wT_r = wT[:]
                        for q in range(4):
                            kt = h * (KT // 2) + j * 4 + q
                            first = kt == 0
                            last = kt == KT - 1
                            lhsT = wT_r[:, q * 128 : (q + 1) * 128]
                            nc.tensor.matmul(
                                acc[:, 0:512],
                                lhsT,
                                xT_r[:, kt * ML : kt * ML + 512],
                                start=first,
                                stop=last,
                            )
                            nc.tensor.matmul(
                                acc[:, 512:1024],
                                lhsT,
                                xT_r[:, kt * ML + 512 : kt * ML + 1024],
                                start=first,
                                stop=last,
                            )
                outsb = outp.tile([128, ML], f32)
                nc.scalar.activation(
                    outsb[:], acc[:], AF.Identity, bias=b_sb[:, nt : nt + 1], scale=1.0
                )
                nc.sync.dma_start(out_t[nt * 128 : (nt + 1) * 128, :], outsb[:])

    _split_waits(nc)
    return nc


def _split_waits(nc, max_waits=1):
    """The walrus build in this container rejects >1 sync-wait per instruction
    ("Too many sync wait commands"). Hoist extra waits onto preceding
    same-engine NOPs, which is semantically identical (in-order engines)."""
    import concourse.mybir as mybir

    for func in nc.m.functions:
        for bb in func.blocks:
            insts = list(bb.instructions)
            new_insts = []
            changed = False
            for inst in insts:
                si = inst.sync_info
                waits = list(si.on_wait) if si is not None and si.on_wait else []
                if len(waits) > max_waits:
                    keep = waits[-max_waits:]
                    for j, wcond in enumerate(waits[:-max_waits]):
                        new_insts.append(
                            mybir.InstNoOp(
                                name=f"{inst.name}-ws{j}",
                                engine=inst.engine,
                                sync_info=mybir.SyncInfo(on_wait=[wcond], on_update=[]),
                            )
                        )
                    si.on_wait = keep
                    inst.sync_info = si
                    changed = True
                new_insts.append(inst)
            if changed:
                bb.instructions = new_insts


def _prep_inputs(x, weight, bias, step_scales):
    x = np.ascontiguousarray(np.asarray(x, dtype=np.float32)).reshape(M, K)
    weight = np.ascontiguousarray(np.asarray(weight, dtype=np.float32))
    bias = np.ascontiguousarray(np.asarray(bias, dtype=np.float32))
    step_scales = np.asarray(step_scales, dtype=np.float32)

    s_eff = (step_scales + np.float32(EPS)).astype(np.float32)      # [G, N]
    recip = (np.float32(1.0) / s_eff).astype(np.float32)            # [G, N]

    def rep(a):  # [G, N] -> [128, NT*G] with col nt*G+g = a[g, nt*128+p]
        return np.ascontiguousarray(
            a.T.reshape(NT, 128, G).transpose(1, 0, 2).reshape(128, NT * G)
        )

    srep = rep(s_eff)
    rrep = rep(recip)
    brep = np.ascontiguousarray(bias.reshape(NT, 128).T)            # [128, NT]
    ident = np.eye(128, dtype=np.float32)

    # one big transpose, then contiguous [K, ML] slices per core
    xt_full = np.ascontiguousarray(x.T)  # [K, M]
    in_maps = []
    for c in range(NCORES):
        in_maps.append(
            {
                "x_t": np.ascontiguousarray(xt_full[:, c * ML : (c + 1) * ML]),
                "w": weight,
                "srep": srep,
                "rrep": rrep,
                "brep": brep,
                "ident": ident,
            }
        )
    return in_maps


def run_on_hw(x, weight, bias, step_scales, trace=False, **kw):
    from concourse.bass_utils import run_bass_kernel_spmd

    if "nc" not in _NC_CACHE:
        _NC_CACHE["nc"] = _build_nc()
    nc = _NC_CACHE["nc"]
    in_maps = _prep_inputs(x, weight, bias, step_scales)
    res = run_bass_kernel_spmd(
        nc, in_maps, core_ids=list(range(NCORES)), trace=trace, **kw
    )
    out_t = np.concatenate([res.results[c]["out_t"] for c in range(NCORES)], axis=1)
    out = np.ascontiguousarray(out_t.T).reshape(B, S, N)
    return out, res


def kernel(x, weight, bias, step_scales):
    out, _ = run_on_hw(x, weight, bias, step_scales, trace=False)
    return out



# revision 3
# speedup vs baseline: 1.3422x; 1.3422x over previous
# kernel.py — nn_CustomLinearEval: group-dequantized linear layer on 8 trn2 cores.
#
# out[b,s,n] = sum_k x[b,s,k] * w_dq[k,n] + bias[n]
#   w_dq = round(weight.T / s) * s,  s = step_scales[g,n] + 1e-8, g = k // 128
#
# Sharding: column-parallel (tensor-parallel over N). Each core owns 512 of the
# 4096 output features:
#   - DMAs its [512, 4096] fp32 weight shard, dequantizes on DVE in natural
#     [n, k] layout (round-half-even via the +/-1.5*2^23 magic trick, matching
#     jnp.round), writing bf16, and transposes the 128 tiles on the PE once.
#     The dequantized w_dq^T (bf16, 4 MiB) stays SBUF-resident.
#   - Streams host-pre-transposed bf16 x^T [K, M] in 16 m-blocks of 512
#     columns (double-buffered), runs pure back-to-back bf16 matmuls
#     (free dim 512) accumulating out^T[n_tile=128, 512] in PSUM over 32
#     k-tiles.
#   - Fuses bias-add into the PSUM->SBUF copy on the scalar engine; out DMAs
#     issue from the scalar engine's DGE so the sync queue never blocks them.
# Host gathers the 8 out^T row-shards and transposes once in numpy.
#
# vs the data-parallel baseline: per-core dequant+transpose work drops 8x
# (128 weight tiles instead of 1024), so the tensor engine runs ~95% pure
# matmul instead of spending ~200us on transposes.

import numpy as np
import ml_dtypes

BF16 = ml_dtypes.bfloat16

GS = 128
EPS = 1e-8
B, S, K, N = 4, 2048, 4096, 4096
M = B * S
NCORES = 8
NL = N // NCORES          # 512 out-features per core
G = K // GS               # 32 quant groups
NT = NL // 128            # 4 n tiles per core
KT = K // 128             # 32 k tiles
MBLK = 512                # columns of x streamed per block
NMBLK = M // MBLK         # 16 m blocks
MAGIC = float(np.float32(12582912.0))  # 1.5 * 2**23: fp32 round-to-nearest-even

_NC_CACHE = {}


def _build_nc():
    import concourse.bass as bass
    import concourse.mybir as mybir
    import concourse.tile as tile

    f32 = mybir.dt.float32
    bf16 = mybir.dt.bfloat16
    AF = mybir.ActivationFunctionType
    OP = mybir.AluOpType

    nc = bass.Bass()
    # x_t: host-pre-transposed x, [K, M] bf16 (pure layout transform on host)
    x_t = nc.dram_tensor("x_t", [K, M], bf16, kind="ExternalInput")
    w = nc.dram_tensor("w", [NL, K], f32, kind="ExternalInput")
    srep = nc.dram_tensor("srep", [128, NT * G], f32, kind="ExternalInput")
    rrep = nc.dram_tensor("rrep", [128, NT * G], f32, kind="ExternalInput")
    brep = nc.dram_tensor("brep", [128, NT], f32, kind="ExternalInput")
    ident = nc.dram_tensor("ident", [128, 128], bf16, kind="ExternalInput")
    out_t = nc.dram_tensor("out_t", [NL, M], f32, kind="ExternalOutput")

    WCH = 1024                # k-columns per weight DMA chunk (8 k-tiles)
    NGRP = KT // 4            # 8 transpose groups of 4 k-tiles per n row

    with tile.TileContext(nc) as tc:
        with (
            tc.tile_pool(name="const", bufs=1) as constp,
            tc.tile_pool(name="wdqT", bufs=1) as wdqTp,
            tc.tile_pool(name="xblk", bufs=2) as xp,
            tc.tile_pool(name="wnat", bufs=2) as wnatp,
            tc.tile_pool(name="t1", bufs=4) as t1p,
            tc.tile_pool(name="wdq", bufs=8) as wdqp,
            tc.tile_pool(name="outsb", bufs=3) as outp,
            tc.tile_pool(name="tp_ps", bufs=2, space="PSUM") as tpps,
            tc.tile_pool(name="acc_ps", bufs=3, space="PSUM") as accps,
        ):
            id_sb = constp.tile([128, 128], bf16)
            nc.sync.dma_start(id_sb[:], ident[:, :])
            s_sb = constp.tile([128, NT * G], f32)
            nc.sync.dma_start(s_sb[:], srep[:, :])
            r_sb = constp.tile([128, NT * G], f32)
            nc.sync.dma_start(r_sb[:], rrep[:, :])
            b_sb = constp.tile([128, NT], f32)
            nc.sync.dma_start(b_sb[:], brep[:, :])

            # persistent dequantized-transposed weight tiles: [k=128, n 4*128]
            # per group of 4 k-tiles; wdqT[nt*NGRP + kt//4][:, (kt%4)*128...]
            wdqT = [
                wdqTp.tile([128, 512], bf16, name=f"wdqT{i}")
                for i in range(NT * NGRP)
            ]

            # x block 0 DMA kicked off before the weight pipeline so the
            # matmul stream can start as soon as the first wdqT row lands.
            xb0 = xp.tile([128, KT * MBLK], bf16, tag="xblk", name="xb0")
            for kt in range(KT):
                nc.sync.dma_start(
                    xb0[:, kt * MBLK : (kt + 1) * MBLK],
                    x_t[kt * 128 : (kt + 1) * 128, 0:MBLK],
                )

            # ---- phase 0: dequant + transpose the weight shard ----
            for nt in range(NT):
                for c in range(K // WCH):
                    wn = wnatp.tile([128, WCH], f32, tag="wnat")
                    nc.sync.dma_start(
                        wn[:], w[nt * 128 : (nt + 1) * 128, c * WCH : (c + 1) * WCH]
                    )
                    for q in range(WCH // 128):
                        kt = c * (WCH // 128) + q
                        col = nt * G + kt
                        grp = nt * NGRP + kt // 4
                        sub = kt % 4
                        t1 = t1p.tile([128, 128], f32)
                        # t1 = (w * (1/s)) + MAGIC  (rounds half-even)
                        nc.vector.tensor_scalar(
                            t1[:],
                            wn[:, q * 128 : (q + 1) * 128],
                            r_sb[:, col : col + 1],
                            MAGIC,
                            op0=OP.mult,
                            op1=OP.add,
                        )
                        # w_dq = (t1 - MAGIC) * s, rounded to bf16 on write
                        wdq = wdqp.tile([128, 128], bf16)
                        nc.vector.tensor_scalar(
                            wdq[:],
                            t1[:],
                            MAGIC,
                            s_sb[:, col : col + 1],
                            op0=OP.subtract,
                            op1=OP.mult,
                        )
                        if sub == 0:
                            ps = tpps.tile([128, 512], bf16)
                        nc.tensor.transpose(
                            ps[:, sub * 128 : (sub + 1) * 128], wdq[:], id_sb[:]
                        )
                        if sub == 3:
                            nc.scalar.copy(wdqT[grp][:], ps[:])

            # ---- phase 1: stream x through pure bf16 matmuls ----
            for mb in range(NMBLK):
                if mb == 0:
                    xb = xb0
                else:
                    xb = xp.tile([128, KT * MBLK], bf16, tag="xblk")
                    m0 = mb * MBLK
                    for kt in range(KT):
                        nc.sync.dma_start(
                            xb[:, kt * MBLK : (kt + 1) * MBLK],
                            x_t[kt * 128 : (kt + 1) * 128, m0 : m0 + MBLK],
                        )
                xb_r = xb[:]
                for nt in range(NT):
                    acc = accps.tile([128, MBLK], f32)
                    for kt in range(KT):
                        grp = nt * NGRP + kt // 4
                        sub = kt % 4
                        nc.tensor.matmul(
                            acc[:],
                            wdqT[grp][:, sub * 128 : (sub + 1) * 128],
                            xb_r[:, kt * MBLK : (kt + 1) * MBLK],
                            start=(kt == 0),
                            stop=(kt == KT - 1),
                        )
                    outsb = outp.tile([128, MBLK], f32)
                    nc.scalar.activation(
                        outsb[:], acc[:], AF.Identity,
                        bias=b_sb[:, nt : nt + 1], scale=1.0,
                    )
                    # out DMA issues from the scalar engine's DGE so it never
                    # queues behind x-block DMA waits on the sync engine.
                    nc.scalar.dma_start(
                        out_t[nt * 128 : (nt + 1) * 128, mb * MBLK : (mb + 1) * MBLK],
                        outsb[:],
                    )

    _split_waits(nc)
    return nc


def _split_waits(nc, max_waits=1):
    """The walrus build in this container rejects >1 sync-wait per instruction
    ("Too many sync wait commands"). Hoist extra waits onto preceding
    same-engine NOPs, which is semantically identical (in-order engines)."""
    import concourse.mybir as mybir

    for func in nc.m.functions:
        for bb in func.blocks:
            insts = list(bb.instructions)
            new_insts = []
            changed = False
            for inst in insts:
                si = inst.sync_info
                waits = list(si.on_wait) if si is not None and si.on_wait else []
                if len(waits) > max_waits:
                    keep = waits[-max_waits:]
                    for j, wcond in enumerate(waits[:-max_waits]):
                        new_insts.append(
                            mybir.InstNoOp(
                                name=f"{inst.name}-ws{j}",
                                engine=inst.engine,
                                sync_info=mybir.SyncInfo(on_wait=[wcond], on_update=[]),
                            )
                        )
                    si.on_wait = keep
                    inst.sync_info = si
                    changed = True
                new_insts.append(inst)
            if changed:
                bb.instructions = new_insts


def _prep_inputs(x, weight, bias, step_scales):
    x = np.ascontiguousarray(np.asarray(x, dtype=np.float32)).reshape(M, K)
    weight = np.ascontiguousarray(np.asarray(weight, dtype=np.float32))
    bias = np.asarray(bias, dtype=np.float32)
    step_scales = np.asarray(step_scales, dtype=np.float32)

    s_eff = (step_scales + np.float32(EPS)).astype(np.float32)      # [G, N]
    recip = (np.float32(1.0) / s_eff).astype(np.float32)            # [G, N]

    xt = np.asarray(x.T, dtype=BF16)                                # [K, M] bf16
    ident = np.eye(128, dtype=BF16)

    def rep(a):  # [G, NL] -> [128, NT*G] with col nt*G+g = a[g, nt*128+p]
        return np.ascontiguousarray(
            a.T.reshape(NT, 128, G).transpose(1, 0, 2).reshape(128, NT * G)
        )

    in_maps = []
    for c in range(NCORES):
        n0 = c * NL
        in_maps.append(
            {
                "x_t": xt,
                "w": np.ascontiguousarray(weight[n0 : n0 + NL, :]),
                "srep": rep(s_eff[:, n0 : n0 + NL]),
                "rrep": rep(recip[:, n0 : n0 + NL]),
                "brep": np.ascontiguousarray(bias[n0 : n0 + NL].reshape(NT, 128).T),
                "ident": ident,
            }
        )
    return in_maps


def run_on_hw(x, weight, bias, step_scales, trace=False, **kw):
    from concourse.bass_utils import run_bass_kernel_spmd

    if "nc" not in _NC_CACHE:
        _NC_CACHE["nc"] = _build_nc()
    nc = _NC_CACHE["nc"]
    in_maps = _prep_inputs(x, weight, bias, step_scales)
    res = run_bass_kernel_spmd(
        nc, in_maps, core_ids=list(range(NCORES)), trace=trace, **kw
    )
    out_t = np.concatenate([res.results[c]["out_t"] for c in range(NCORES)], axis=0)
    out = np.ascontiguousarray(out_t.T).reshape(B, S, N)
    return out, res


def kernel(x, weight, bias, step_scales):
    out, _ = run_on_hw(x, weight, bias, step_scales, trace=False)
    return out
